# revision 70
# baseline (speedup 1.0000x reference)
"""Trainium2 Bass kernel for nn_C2f_DualModal_MoE (v6: fp8 DoubleRow experts).

Full inputs in, full outputs out. Data-parallel over batch: 16 items on
8 cores (2 per core).

Precision plan (rel-err budget 2e-2, measured ~1.6e-2 in numpy model):
  - cv1 / cv2 matmuls in bf16 (1 col/cycle on PE, quant error ~0.2%)
  - expert 3x3 convs in fp8 e4m3 with MatmulPerfMode.DoubleRow: each
    matmul contracts 256 rows (two taps of 128 channels) at 0.5
    cycles/column -> 4x the f32r rate. 9 taps = 5 pairs (last zero-padded).
  - m stored twice: bf16 padded (cv2 rhs) + fp8 padded copy (expert rhs,
    converted on gpsimd), sa/s/moe bf16.
  - routing (softmax top-2) exact f32 as before; gates folded into the
    DVE moe combine (moe = g0*s0 + g1*s1, bf16 2x mode).

Engine budget per core: PE ~80us, Act ~86us (bottleneck: 96 silu
evacuations of 800 cols), DVE/Pool/SP under 55us each.
"""

import sys

for _p in ("/opt/trn_rl_repo", "/opt/pypackages"):
    if _p not in sys.path:
        sys.path.append(_p)

import numpy as np
import ml_dtypes
import concourse.bass as bass
import concourse.mybir as mybir
import concourse.tile as tile
from concourse import bacc
from concourse.bass import ds, AP
from concourse.bass_utils import run_bass_kernel_spmd

F32 = mybir.dt.float32
BF16 = mybir.dt.bfloat16
FP8 = mybir.dt.float8e4
AF = mybir.ActivationFunctionType
DR = mybir.MatmulPerfMode.DoubleRow
ALU = mybir.AluOpType

N_CORES = 8
B = 16
BPC = B // N_CORES
C1 = 256
C = 128
E = 4
H = W = 80
S = H * W  # 6400
R = 5  # image rows per spatial tile
N = R * W  # 400 matmul columns per tile
NT = H // R  # 16 tiles
NG = NT // 2  # 8 groups of 2 tiles
GN = 2 * N  # 800 cols per group
HP = H + 2  # 82
HP8 = HP + 1  # 83: fp8 m copy has spare row/col so tap-pair APs stay in-bounds
NV = R * HP8  # 415: expert matmul cols per tile (80 valid per 83, rest junk)
WSCALE = 64.0  # host scale on expert weights before e4m3 quantization
INV_S = 1.0 / S
# 3x3 taps as (dy, dx), paired for DoubleRow; 9th tap pairs with zeros
TAP_PAIRS = [
    ((0, 0), (0, 1)),
    ((0, 2), (1, 0)),
    ((1, 1), (1, 2)),
    ((2, 0), (2, 1)),
    ((2, 2), (2, 2)),  # second half has zero weights; AP uses stride +1
]

_cache = {}


def _build_program(reps=1):
    nc = bacc.Bacc(
        "TRN2",
        target_bir_lowering=False,
        debug=False,
        enable_asserts=True,
        dynamic_dma_scratch_size=4096,
    )
    x_d = nc.dram_tensor("x", [BPC, 2, C, S], BF16, kind="ExternalInput").ap()
    w1_d = nc.dram_tensor("w1", [C, 2, 2 * C], BF16, kind="ExternalInput").ap()
    b1_d = nc.dram_tensor("b1", [2 * C, 1], F32, kind="ExternalInput").ap()
    wr_d = nc.dram_tensor("wrt", [C, E], F32, kind="ExternalInput").ap()
    br_d = nc.dram_tensor("br", [1, E], F32, kind="ExternalInput").ap()
    wexp_d = nc.dram_tensor("wexp", [E * C, 5, 2, C], FP8, kind="ExternalInput").ap()
    bexp_d = nc.dram_tensor("bexp", [E * C, 1], F32, kind="ExternalInput").ap()
    w2_d = nc.dram_tensor("w2", [3 * C, 2 * C], BF16, kind="ExternalInput").ap()
    b2_d = nc.dram_tensor("b2", [2 * C, 1], F32, kind="ExternalInput").ap()
    out_d = nc.dram_tensor("out", [BPC, 2 * C, S], F32, kind="ExternalOutput").ap()

    with tile.TileContext(nc) as tc:
        _emit(nc, tc, x_d, w1_d, b1_d, wr_d, br_d, wexp_d, bexp_d, w2_d, b2_d, out_d, reps)
    nc.compile()
    return nc


def _emit(nc, tc, x_d, w1_d, b1_d, wr_d, br_d, wexp_d, bexp_d, w2_d, b2_d, out_d, reps=1):
    from contextlib import ExitStack

    ctx = ExitStack()
    with ctx:
        wp = ctx.enter_context(tc.tile_pool(name="weights", bufs=1))
        sp = ctx.enter_context(tc.tile_pool(name="stream", bufs=2))
        pp = ctx.enter_context(tc.tile_pool(name="psum", bufs=1, space="PSUM"))

        # --- static weights -------------------------------------------------
        # w1 leads the SP queue; b1 trails the first x half-chunk on Pool
        # (emitted from p1_group); late-needed weights (w2/b2/wrt/br) are
        # DMA'd on SP after item 1's first x chunk (see late_weights()).
        w1 = wp.tile([C, 2, 2 * C], BF16, name="w1sb")
        nc.sync.dma_start(w1[:], w1_d)
        b1 = wp.tile([C, 2], F32, name="b1sb")

        def b1_dma():
            for mt in range(2):
                nc.gpsimd.dma_start(b1[:, mt : mt + 1], b1_d[mt * C : (mt + 1) * C, :])
        wrt = wp.tile([C, E], F32, name="wrtsb")
        br = wp.tile([1, E], F32, name="brsb")
        w2c = [wp.tile([C, 2 * C], BF16, name=f"w2c{j}") for j in range(3)]
        b2 = wp.tile([C, 2], F32, name="b2sb")

        def late_weights():
            nc.sync.dma_start(wrt[:], wr_d)
            nc.sync.dma_start(br[:], br_d)
            for j in range(3):
                nc.sync.dma_start(w2c[j][:], w2_d[j * C : (j + 1) * C, :])
            for mt in range(2):
                nc.sync.dma_start(b2[:, mt : mt + 1], b2_d[mt * C : (mt + 1) * C, :])

        ones = wp.tile([1, C], F32, name="ones")
        nc.vector.memset(ones[:], 1.0)
        zrow = wp.tile([C, HP], BF16, name="zrow")
        nc.vector.memset(zrow[:], 0.0)
        # trigger the silu activation-table load at t~0 instead of right
        # before the first real activation (saves its 1.3us from the path)
        atl = wp.tile([1, 1], F32, name="atl")
        nc.scalar.activation(atl[:], ones[0:1, 0:1], AF.Silu)

        # all-4 expert weights staged in SBUF; routing then selects with a
        # DVE copy at a dynamic offset (no DMA latency / queue coupling)
        wall = wp.tile([C, E * 1280], FP8, name="wall")
        ball = wp.tile([C, E], F32, name="ball")

        def prefetch_experts():
            for e in range(E):
                nc.gpsimd.dma_start(
                    wall[:, e * 1280 : (e + 1) * 1280],
                    wexp_d[e * C : (e + 1) * C, :, :, :],
                )
                nc.gpsimd.dma_start(
                    ball[:, e : e + 1], bexp_d[e * C : (e + 1) * C, :]
                )

        def p1_state(b):
            sa = sp.tile([C, S], BF16, tag="sa", bufs=2)
            mp = sp.tile([C, HP, HP], BF16, tag="mp", bufs=2)
            mp8 = sp.tile([C, HP8, HP8], FP8, tag="mp8", bufs=2)
            parts = sp.tile([C, NG], F32, tag="parts", bufs=2)
            nc.vector.tensor_copy(mp[:, 0:1, :], zrow[:, None, :])
            nc.vector.tensor_copy(mp[:, HP - 1 : HP, :], zrow[:, None, :])
            nc.vector.tensor_copy(mp[:, 1 : HP - 1, 0:1], zrow[:, 0 : HP - 2, None])
            nc.vector.tensor_copy(
                mp[:, 1 : HP - 1, HP - 1 : HP], zrow[:, 0 : HP - 2, None]
            )
            # zero the spare row/col of the fp8 copy (read as junk columns
            # by the full-row expert matmuls; must be finite)
            nc.gpsimd.memset(mp8[:, :, HP : HP + 1], 0.0)
            nc.gpsimd.memset(mp8[:, HP : HP + 1, :], 0.0)
            return sa, mp, mp8, parts

        def part_reduce(mp, parts, g):
            # pooled partial sum on DVE; rows are contiguous incl. zero pad
            # cols, so flatten
            base = mp[:, 1 + 10 * g, 0:1]
            flat = AP(base.tensor, base.offset, [base.ap[0], [1, 10 * HP]])
            nc.vector.tensor_reduce(
                parts[:, g : g + 1], flat, op=ALU.add, axis=mybir.AxisListType.X
            )

        def p1_group(b, g, st, xs):
            """cv1 (bf16) for spatial tiles 2g, 2g+1."""
            sa, mp, mp8, parts = st
            chunk = g // 2
            if g % 2 == 0 and xs[chunk] is None:
                xg = sp.tile([C, 2, 4 * N], BF16, tag="x", bufs=4, name=f"x{b}_{chunk}")
                # item 0's chunks split across the Pool/SP queues so the
                # startup isn't starved behind a single serial DMA queue
                for k in range(2):
                    eng = nc.gpsimd if (b == 0 and k == 0) else nc.sync
                    eng.dma_start(
                        xg[:, k, :], x_d[b, k, :, chunk * 4 * N : (chunk + 1) * 4 * N]
                    )
                    if b == 0 and chunk == 0 and k == 0:
                        b1_dma()
                xs[chunk] = xg
            xg = xs[chunk]
            off = (g % 2) * GN
            ps_a = pp.tile([C, 2, 512], F32, tag="ps", bufs=4, name=f"ps1a_{b}_{g}")
            ps_m = pp.tile([C, 2, 512], F32, tag="ps", bufs=4, name=f"ps1m_{b}_{g}")
            pss = [ps_a, ps_m]
            for k in range(2):
                for mt in range(2):
                    ms = slice(mt * C, (mt + 1) * C)
                    for i in range(2):
                        nc.tensor.matmul(
                            pss[mt][:, i, 0:N],
                            w1[:, k, ms],
                            xg[:, k, off + i * N : off + (i + 1) * N],
                            start=(k == 0),
                            stop=(k == 1),
                        )
            # a branch: silu -> sa (bf16, flat)
            nc.scalar.activation(
                sa[:, g * GN : (g + 1) * GN],
                ps_a[:, :, 0:N],
                AF.Silu,
                bias=b1[:, 0:1],
            )
            # m branch: silu -> mp (bf16, padded). Pooled partials: item 1's
            # (and item 0's LAST group's) come from the Act accumulator
            # (+187ns each) -- this makes parts complete the instant the act
            # ends, and leaves no deferred DVE reduces to congest the DVE
            # queue during routing(0)'s latency-critical chain. Item 0's
            # earlier groups use DVE reduces (idle there) to keep Act light.
            use_accum = b == 1 or g == NG - 1
            nc.scalar.activation(
                mp[:, 1 + 10 * g : 11 + 10 * g, 1 : 1 + W],
                ps_m[:, :, 0:N],
                AF.Silu,
                bias=b1[:, 1:2],
                accum_out=(parts[:, g : g + 1] if use_accum else None),
            )
            if not use_accum:
                part_reduce(mp, parts, g)
            # fp8 copy of the padded m rows for the expert convs (gpsimd).
            # Group g covers padded rows [1+10g, 11+10g); extend to the
            # border rows at the ends.
            r0 = 0 if g == 0 else 1 + 10 * g
            r1 = HP if g == NG - 1 else 11 + 10 * g
            nc.gpsimd.tensor_copy(
                mp8[:, r0:r1, 0:HP], mp[:, r0:r1, :]
            )

        def routing(b, st):
            with tc.high_priority():
                return _routing(b, st)

        def _routing(b, st):
            sa, mp, mp8, parts = st
            pooled = sp.tile([C, 1], F32, tag="pooled", bufs=2)
            nc.vector.reduce_sum(pooled[:], parts[:], axis=mybir.AxisListType.X)
            # logits in [1, E] layout directly (pooled as stationary operand)
            # so no partition-transpose DMA is needed before the top-2 scan
            ps_l = pp.tile([1, E], F32, tag="ps", bufs=4)
            nc.tensor.matmul(ps_l[:], pooled[:], wrt[:], start=True, stop=True)
            row = sp.tile([1, 8], F32, tag="row", bufs=2)
            nc.vector.memset(row[:], -1e30)
            nc.vector.tensor_scalar(row[0:1, 0:E], ps_l[:], INV_S, None, op0=ALU.mult)
            nc.vector.tensor_tensor(row[0:1, 0:E], row[0:1, 0:E], br[:], op=ALU.add)
            vals = sp.tile([1, 8], F32, tag="vals", bufs=2)
            nc.vector.max(vals[:], row[:])
            uidx = sp.tile([1, 8], mybir.dt.uint32, tag="uidx", bufs=2)
            nc.vector.max_index(uidx[:], vals[:], row[:])
            # gates: g0 = sigmoid(l0 - l1) = silu(d)/d, g1 = 1 - g0
            scr = sp.tile([1, 4], F32, tag="scr", bufs=2)
            nc.vector.tensor_tensor(
                scr[:, 0:1], vals[:, 0:1], vals[:, 1:2], op=ALU.subtract
            )
            nc.vector.reciprocal(scr[:, 1:2], scr[:, 0:1])
            nc.scalar.activation(scr[:, 2:3], scr[:, 0:1], AF.Silu)
            g = sp.tile([1, 2], F32, tag="g", bufs=2)
            nc.vector.tensor_tensor(g[:, 0:1], scr[:, 2:3], scr[:, 1:2], op=ALU.mult)
            nc.vector.tensor_scalar(
                g[:, 1:2], g[:, 0:1], -1.0, 1.0, op0=ALU.mult, op1=ALU.add
            )
            ps_g = pp.tile([C, 2], F32, tag="ps", bufs=4)
            nc.tensor.matmul(ps_g[:], ones[:], g[:], start=True, stop=True)
            g_bc = sp.tile([C, 2], F32, tag="gbc", bufs=2)
            nc.vector.tensor_copy(g_bc[:], ps_g[:])
            wks = []
            for k in range(2):
                iv = nc.values_load(
                    uidx[0:1, k : k + 1],
                    min_val=0,
                    max_val=E - 1,
                    skip_runtime_bounds_check=True,
                )
                wk = sp.tile([C, 1280], FP8, tag=f"expw{k}", bufs=2, name=f"expw{k}")
                bk = sp.tile([C, 1], F32, tag=f"expb{k}", bufs=2, name=f"expb{k}")
                eng = nc.vector if k == 0 else nc.gpsimd
                eng.tensor_copy(wk[:], wall[:, ds(iv * 1280, 1280)])
                eng.tensor_copy(bk[:], ball[:, ds(iv, 1)])
                wks.append((wk, bk))
            return wks, g_bc

        def pair_lhs(wk, p):
            """[C, 2, C] fp8 lhsT view of the flat gathered expert blob."""
            base = wk[:, p * 2 * C : p * 2 * C + 1]
            return AP(base.tensor, base.offset, [base.ap[0], [C, 2], [1, C]])

        def pair_rhs(mp8, t, pair):
            """[C, 2, NV] fp8 AP: two flat full-row windows of the m copy."""
            (y0, x0), (y1, x1) = pair
            r0 = t * R
            base = mp8[:, r0 + y0, x0 : x0 + 1]
            d = (y1 - y0) * HP8 + (x1 - x0)
            if pair[0] == pair[1]:
                d = 1  # zero-weight half; any in-bounds stride works
            return AP(base.tensor, base.offset, [base.ap[0], [d, 2], [1, NV]])

        def valid_cols(ps, bank0):
            """[C, 2, R, W] f32 AP over the valid columns of two PSUM banks."""
            base = ps[:, bank0, 0:1]
            return AP(
                base.tensor, base.offset, [base.ap[0], [512, 2], [HP8, R], [1, W]]
            )

        def exp_group(b, g, st, rt):
            sa, mp, mp8, parts = st
            wks, g_bc = rt
            pse = [
                pp.tile([C, 2, 512], F32, tag="ps", bufs=4, name=f"pse{k}_{b}_{g}")
                for k in range(2)
            ]
            for k in range(2):
                for i in range(2):
                    t = 2 * g + i
                    for p in range(5):
                        nc.tensor.matmul(
                            pse[k][:, i, 0:NV],
                            pair_lhs(wks[k][0], p),
                            pair_rhs(mp8, t, TAP_PAIRS[p]),
                            start=(p == 0),
                            stop=(p == 4),
                            perf_mode=DR,
                        )
            sg = sp.tile([C, 2, GN], BF16, tag="sg", bufs=3, name=f"sg{b}_{g}")
            for k in range(2):
                nc.scalar.activation(
                    sg[:, k, :],
                    valid_cols(pse[k], 0),
                    AF.Silu,
                    bias=wks[k][1][:],
                    scale=1.0 / WSCALE,
                )
            moe = sp.tile([C, GN], BF16, tag="moe", bufs=3, name=f"moe{b}_{g}")
            nc.vector.tensor_scalar_mul(moe[:], sg[:, 0, :], g_bc[:, 0:1])
            nc.vector.scalar_tensor_tensor(
                moe[:], sg[:, 1, :], g_bc[:, 1:2], moe[:], op0=ALU.mult, op1=ALU.add
            )
            return moe

        def cv2_group(b, g, st, moe, last=False):
            sa, mp, mp8, parts = st
            pso = [
                pp.tile([C, 2, 512], F32, tag="ps", bufs=4, name=f"pso{mt}_{b}_{g}")
                for mt in range(2)
            ]
            for mt in range(2):
                ms = slice(mt * C, (mt + 1) * C)
                for i in range(2):
                    t = 2 * g + i
                    dst = pso[mt][:, i, 0:N]
                    nc.tensor.matmul(
                        dst, w2c[0][:, ms], sa[:, t * N : (t + 1) * N],
                        start=True, stop=False,
                    )
                    nc.tensor.matmul(
                        dst, w2c[1][:, ms],
                        mp[:, 1 + t * R : 1 + (t + 1) * R, 1 : 1 + W],
                        start=False, stop=False,
                    )
                    nc.tensor.matmul(
                        dst, w2c[2][:, ms], moe[:, i * N : (i + 1) * N],
                        start=False, stop=True,
                    )
            # the final group splits its acts/stores into 400-col pieces so
            # the last DMA overlaps the last activation instead of trailing
            # a full 800-col one
            nh = 2 if last else 1
            for mt in range(2):
                ms = slice(mt * C, (mt + 1) * C)
                for h in range(2 // nh * 0 + nh):
                    w0 = g * GN + h * (GN // nh)
                    ot = sp.tile(
                        [C, GN // nh], F32, tag=f"ot{mt}", bufs=3, name=f"ot{mt}_{g}_{h}"
                    )
                    nc.scalar.activation(
                        ot[:],
                        pso[mt][:, h * (2 // nh) : (h + 1) * (2 // nh), 0:N],
                        AF.Silu,
                        bias=b2[:, mt : mt + 1],
                    )
                    # item 0's outs all go on SP so Pool's queue stays short
                    # for the expert weight gathers (in-order queues; a
                    # gather stuck behind out DMAs stalls the expert
                    # matmuls). By item 1's p2 phase Pool is idle, so its
                    # outs split across both queues (and the final group goes
                    # entirely to Pool, which is empty by then).
                    if last:
                        eng = nc.gpsimd if (mt + h) % 2 == 0 else nc.sync
                    elif b == 1 and mt == 1:
                        eng = nc.gpsimd
                    else:
                        eng = nc.sync
                    eng.dma_start(out_d[b, ms, w0 : w0 + GN // nh], ot[:])

        for _rep in range(reps):
            xs = [[None] * 4, [None] * 4]
            st = [None, None]
            st[0] = p1_state(0)
            for g in range(NG):
                p1_group(0, g, st[0], xs[0])
            prefetch_experts()
            st[1] = p1_state(1)
            p1_group(1, 0, st[1], xs[1])
            late_weights()
            p1_group(1, 1, st[1], xs[1])
            rt = [routing(0, st[0]), None]
            # p2: one continuous software pipeline over both items' groups;
            # cv2 lags exp by one group so the exp->act->moe chain of a group
            # resolves while the PE runs cv2 of the previous one. The
            # remaining p1(1) groups interleave as PE fillers, and their acts
            # are deliberately spread so the exp acts aren't stuck behind
            # them in the in-order Act queue.
            pairs = [(b, g) for b in range(BPC) for g in range(NG)]
            moes = [None] * len(pairs)
            nxt = 2
            for j in range(len(pairs) + 1):
                nfill = 3 if j == 0 else 1
                for _ in range(nfill):
                    if nxt < NG:
                        p1_group(1, nxt, st[1], xs[1])
                        nxt += 1
                if j < len(pairs):
                    b, g = pairs[j]
                    moes[j] = exp_group(b, g, st[b], rt[b])
                if j >= 1:
                    b, g = pairs[j - 1]
                    cv2_group(b, g, st[b], moes[j - 1], last=(j == len(pairs)))
                if j == 4:
                    rt[1] = routing(1, st[1])


def _prep_inputs(x, W_cv1, b_cv1, W_r, b_r, W_exp, b_exp, W_cv2, b_cv2):
    """Host-side packing shared by kernel() and the test harness."""
    BF = ml_dtypes.bfloat16
    E4 = ml_dtypes.float8_e4m3fn
    x = np.asarray(x, np.float32)
    # x: [B, C1, S] -> [B, 2, 128, S] bf16
    xp = np.ascontiguousarray(x.reshape(B, 2, C, S).astype(BF))
    # w1: [256out, 256in] -> [p, k, out]
    w1 = np.asarray(W_cv1, np.float32)[:, :, 0, 0]  # [2C out, C1 in]
    w1p = np.ascontiguousarray(w1.T.reshape(2, C, 2 * C).transpose(1, 0, 2).astype(BF))
    # expert weights: [E, out, in, ky, kx] scaled, e4m3, packed [E*C, 5, 2, C]
    we = np.asarray(W_exp, np.float32) * WSCALE
    weq = we.astype(E4)
    wexp = np.zeros((E, C, 5, 2, C), E4)
    for p, (t0, t1) in enumerate(TAP_PAIRS):
        wexp[:, :, p, 0, :] = weq[:, :, :, t0[0], t0[1]].transpose(0, 2, 1)
        if p < 4:
            wexp[:, :, p, 1, :] = weq[:, :, :, t1[0], t1[1]].transpose(0, 2, 1)
    wexp = np.ascontiguousarray(wexp.reshape(E * C, 5, 2, C))
    bexp = np.ascontiguousarray(
        np.asarray(b_exp, np.float32).reshape(E * C, 1)
    )
    w2 = np.asarray(W_cv2, np.float32)[:, :, 0, 0]  # [256 out, 384 in]
    w2p = np.ascontiguousarray(w2.T.astype(BF))  # [384, 256]
    shared = {
        "w1": w1p,
        "b1": np.asarray(b_cv1, np.float32).reshape(-1, 1),
        "wrt": np.ascontiguousarray(np.asarray(W_r, np.float32).T),
        "br": np.asarray(b_r, np.float32).reshape(1, E),
        "wexp": wexp,
        "bexp": bexp,
        "w2": w2p,
        "b2": np.asarray(b_cv2, np.float32).reshape(-1, 1),
    }
    return [
        {**shared, "x": np.ascontiguousarray(xp[i * BPC : (i + 1) * BPC])}
        for i in range(N_CORES)
    ]


def kernel(x, W_cv1, b_cv1, W_r, b_r, W_exp, b_exp, W_cv2, b_cv2):
    if "nc" not in _cache:
        _cache["nc"] = _build_program()
    nc = _cache["nc"]
    in_maps = _prep_inputs(x, W_cv1, b_cv1, W_r, b_r, W_exp, b_exp, W_cv2, b_cv2)
    res = run_bass_kernel_spmd(nc, in_maps, core_ids=list(range(N_CORES)))
    _cache["last_results"] = res
    out = np.concatenate([res.results[i]["out"] for i in range(N_CORES)], axis=0)
    return out.reshape(B, 2 * C, H, W)
